# revision 15
# baseline (speedup 1.0000x reference)
"""Trainium2 Bass kernel for nn_LstmCrf: bidirectional LSTM + CRF log-partition.

Contract: kernel(**inputs) takes the FULL unsharded inputs (see shapes below) and
returns the FULL output logZ [128] f32. Internally shards the batch (128 rows)
across 8 NeuronCores (16 rows each), runs one SPMD Bass/Tile program, and
concatenates the per-core results.

Problem shapes (hardcoded): B=128, T=512, V=50000, E=100, U=128, K=32.

Per-core device program:
  1. Embedding gather via indirect DMA (tokens staged t-major), PE-transpose to
     x_T [104, T*16] bf16 (E padded to 104; col 100 carries 1.0 so the LSTM bias
     rides row 100 of the augmented Wk).
  2. Bidirectional LSTM scans, fwd+bwd interleaved per step; gates via one
     sigmoid + one tanh ACT op per step (gate blocks pre-permuted to i,f,o,g);
     h stored bf16.
  3. Emissions em = h_f@Ck_f + h_b@Ck_b; em_e = exp(em + crf_bias - delta) bf16.
  4. CRF forward DP in the exp domain (alpha_t = (Ae^T alpha) * em_e_t with
     Ae = exp(trans)), run meet-in-the-middle from both ends;
     logZ = log(sum_j alpha_mid * beta_mid) + T*delta,  delta = log(K).
"""
import sys
from contextlib import ExitStack

import numpy as np

for p in ("/opt/trn_rl_repo", "/root/.axon_site/_ro/trn_rl_repo"):
    if p not in sys.path:
        sys.path.append(p)

import ml_dtypes

NPBF16 = ml_dtypes.bfloat16

B, T = 128, 512
V, E, U, K = 50000, 100, 128, 32
NCORES = 8
BL = B // NCORES          # 16 rows per core
EA = 104                  # padded embedding dim
G4 = 4 * U
DELTA = float(np.log(K))


def _build_program(T=T):
    import concourse.bacc as bacc
    import concourse.bass as bass
    import concourse.mybir as mybir
    import concourse.tile as tile

    F32 = mybir.dt.float32
    BF16 = mybir.dt.bfloat16
    I32 = mybir.dt.int32
    AF = mybir.ActivationFunctionType
    ALU = mybir.AluOpType

    NBLK = T * BL // 128
    MID = T // 2

    nc = bacc.Bacc(None, target_bir_lowering=False, debug=False)

    tok = nc.dram_tensor("tok", [128, NBLK], I32, kind="ExternalInput")
    emb = nc.dram_tensor("emb", [V, EA], F32, kind="ExternalInput")
    wk_f = nc.dram_tensor("wk_f", [EA, G4], BF16, kind="ExternalInput")
    wk_b = nc.dram_tensor("wk_b", [EA, G4], BF16, kind="ExternalInput")
    wr_f = nc.dram_tensor("wr_f", [U, G4], BF16, kind="ExternalInput")
    wr_b = nc.dram_tensor("wr_b", [U, G4], BF16, kind="ExternalInput")
    ck_f = nc.dram_tensor("ck_f", [U, K], BF16, kind="ExternalInput")
    ck_b = nc.dram_tensor("ck_b", [U, K], BF16, kind="ExternalInput")
    ae = nc.dram_tensor("ae", [K, K], F32, kind="ExternalInput")
    aet = nc.dram_tensor("aet", [K, K], F32, kind="ExternalInput")
    embias = nc.dram_tensor("embias", [K, 1], F32, kind="ExternalInput")
    ident = nc.dram_tensor("ident", [128, 128], F32, kind="ExternalInput")
    out = nc.dram_tensor("out", [1, BL], F32, kind="ExternalOutput")

    def block_order(nblk):
        order = []
        lo, hi = 0, nblk - 1
        while lo <= hi:
            order.append(lo)
            if hi != lo:
                order.append(hi)
            lo += 1
            hi -= 1
        return order

    with tile.TileContext(nc) as tc, ExitStack() as ctx:
        P = ctx.enter_context(tc.tile_pool(name="persist", bufs=1))
        tok_t = P.tile([128, NBLK], I32, tag="tok")
        wkf_t = P.tile([EA, G4], BF16, tag="wkf")
        wkb_t = P.tile([EA, G4], BF16, tag="wkb")
        wrf_t = P.tile([U, G4], BF16, tag="wrf")
        wrb_t = P.tile([U, G4], BF16, tag="wrb")
        ckf_t = P.tile([U, K], BF16, tag="ckf")
        ckb_t = P.tile([U, K], BF16, tag="ckb")
        ae_t = P.tile([K, K], F32, tag="ae")
        aet_t = P.tile([K, K], F32, tag="aet")
        embias_t = P.tile([K, 1], F32, tag="embias")
        ident_t = P.tile([128, 128], F32, tag="ident")
        xT = P.tile([EA, T * BL], BF16, tag="xT")
        h_all = P.tile([U, 2 * T * BL], BF16, tag="hall")
        em_e = P.tile([K, T * BL], BF16, tag="eme")
        ones_t = P.tile([K, 1], F32, tag="ones")

        nc.sync.dma_start(tok_t[:], tok[:])
        nc.sync.dma_start(wkf_t[:], wk_f[:])
        nc.sync.dma_start(wkb_t[:], wk_b[:])
        nc.sync.dma_start(wrf_t[:], wr_f[:])
        nc.sync.dma_start(wrb_t[:], wr_b[:])
        nc.sync.dma_start(ckf_t[:], ck_f[:])
        nc.sync.dma_start(ckb_t[:], ck_b[:])
        nc.sync.dma_start(ae_t[:], ae[:])
        nc.sync.dma_start(aet_t[:], aet[:])
        nc.sync.dma_start(embias_t[:], embias[:])
        nc.sync.dma_start(ident_t[:], ident[:])
        nc.vector.memset(ones_t[:], 1.0)

        with (
            tc.tile_pool(name="gat", bufs=4) as gat,
            tc.tile_pool(name="tp_ps", bufs=2, space="PSUM") as tp_ps,
            tc.tile_pool(name="zps", bufs=6, space="PSUM") as zps,
            tc.tile_pool(name="sg", bufs=4) as sgp,
            tc.tile_pool(name="cst", bufs=4) as cst,
        ):
            for k in block_order(NBLK):
                g = gat.tile([128, EA], F32, tag="g")
                nc.gpsimd.indirect_dma_start(
                    out=g[:],
                    out_offset=None,
                    in_=emb[:],
                    in_offset=bass.IndirectOffsetOnAxis(ap=tok_t[:, k:k + 1], axis=0),
                )
                pt = tp_ps.tile([EA, 128], F32, tag="pt")
                nc.tensor.transpose(pt[:], g[:], ident_t[:])
                nc.vector.tensor_copy(xT[:, k * 128:(k + 1) * 128], pt[:])

            # LSTM scans.
            # PSUM z layout per step: [i_f f_f o_f g2_f | i_b f_b o_b g2_b]
            # (g2 = pre-doubled g gate; host scaled its weights by 2).
            # sg = sigmoid(z) on all 128 cols in ONE ACT op; tanh(g) = 2*sg(g2)-1.
            # State tile X_t [128, 2, 32] per dir: [tg_t (16) | c_{t-1} (16)].
            # prods = sg[i|f] * [tg | c]; c_t = prods[:16] + prods[16:32].
            c_prev = None
            for t in range(T):
                z = zps.tile([128, 128], F32, tag="z")
                # emit all x-projection MMs first: they depend only on xT, so
                # the PE FIFO can run them during the previous step's ACT/DVE
                # phase instead of stalling them behind h-dependent Wr MMs.
                # Gate-major z layout: gate g at cols [g*32,(g+1)*32), fwd dir
                # at +0, bwd at +16 -> sigma slices are contiguous [128,32].
                # One accumulation group per z tile: start=True on the FIRST MM
                # zeroes the whole 2KB bank; everything else accumulates.
                # x-projection MMs are emitted first so the PE FIFO runs them
                # during the previous step's ACT/DVE phase.
                first = True
                for d, wk_t in ((0, wkf_t), (1, wkb_t)):
                    tt = t if d == 0 else T - 1 - t
                    xs = xT[:, tt * BL:(tt + 1) * BL]
                    for gi in range(4):
                        oc = gi * 32 + d * BL
                        nc.tensor.matmul(
                            z[:, oc:oc + BL],
                            wk_t[:, gi * U:(gi + 1) * U],
                            xs,
                            start=first,
                            stop=(t == 0 and d == 1 and gi == 3),
                        )
                        first = False
                if t > 0:
                    for d, (wr_t, hofs) in ((0, (wrf_t, 0)), (1, (wrb_t, T * BL))):
                        hprev = t - 1 if d == 0 else T - t
                        hs = h_all[:, hofs + hprev * BL:hofs + (hprev + 1) * BL]
                        for gi in range(4):
                            oc = gi * 32 + d * BL
                            nc.tensor.matmul(
                                z[:, oc:oc + BL],
                                wr_t[:, gi * U:(gi + 1) * U],
                                hs,
                                start=False,
                                stop=(d == 1 and gi == 3),
                            )
                sg = sgp.tile([128, 128], F32, tag="sg")
                nc.scalar.activation(sg[:], z[:], AF.Sigmoid)
                # si = sg[0:32], sf = sg[32:64], so = sg[64:96], sgg = sg[96:128]
                # c = sf*c_prev + si*tanh(g), tanh(g) = 2*sg(g2)-1:
                #   m1 = si*sgg; m2 = sf*c_prev; m3 = m2 - si; c = 2*m1 + m3
                # c = sf*c_prev + si*(2*sg(g2)-1):
                #   m1 = si*sgg; m2 = sf*c_prev; w = 2*m1 - si; c = w + m2
                # (m1, m2 independent; w depends on m1 two issues back -> only
                #  the final add pays a same-engine RAW stall)
                m1 = cst.tile([128, 32], F32, tag="m1")
                nc.vector.tensor_tensor(m1[:], sg[:, 0:32], sg[:, 96:128], ALU.mult)
                if t == 0:
                    c_new = cst.tile([128, 32], F32, tag="c")
                    nc.vector.scalar_tensor_tensor(
                        c_new[:], m1[:], 2.0, sg[:, 0:32], ALU.mult, ALU.subtract)
                else:
                    m2 = cst.tile([128, 32], F32, tag="m2")
                    nc.vector.tensor_tensor(m2[:], sg[:, 32:64], c_prev[:], ALU.mult)
                    w = cst.tile([128, 32], F32, tag="w")
                    nc.vector.scalar_tensor_tensor(
                        w[:], m1[:], 2.0, sg[:, 0:32], ALU.mult, ALU.subtract)
                    c_new = cst.tile([128, 32], F32, tag="c")
                    nc.vector.tensor_tensor(c_new[:], w[:], m2[:], ALU.add)
                c_prev = c_new
                tct = cst.tile([128, 32], F32, tag="tc")
                nc.scalar.activation(tct[:], c_new[:], AF.Tanh)
                # single h op for both dirs: 2-block out AP into h_all
                # (block 0 = fwd slot t, block 1 = bwd slot T-1-t)
                p0 = t * BL
                p1 = T * BL + (T - 1 - t) * BL
                hb = h_all[:, p0:p0 + BL]
                hout = bass.AP(hb.tensor, hb.offset,
                               [hb.ap[0], [p1 - p0, 2], [1, BL]])
                nc.vector.tensor_tensor(hout, sg[:, 64:96], tct[:], ALU.mult)

        # keep the exp/ln table phase strictly after the sigmoid/tanh phase
        tc.no_sync_barrier()

        EMC = 512
        with (
            tc.tile_pool(name="emps", bufs=4, space="PSUM") as emps,
            tc.tile_pool(name="crf", bufs=3) as crf,
            tc.tile_pool(name="crfps", bufs=2, space="PSUM") as crfps,
        ):
            for ch in range(T * BL // EMC):
                ep = emps.tile([K, EMC], F32, tag="ep")
                nc.tensor.matmul(ep[:], ckf_t[:], h_all[:, ch * EMC:(ch + 1) * EMC],
                                 start=True, stop=False)
                nc.tensor.matmul(ep[:], ckb_t[:],
                                 h_all[:, T * BL + ch * EMC:T * BL + (ch + 1) * EMC],
                                 start=False, stop=True)
                nc.scalar.activation(em_e[:, ch * EMC:(ch + 1) * EMC], ep[:],
                                     AF.Exp, bias=embias_t[:], scale=1.0)

            a_cur = crf.tile([K, BL], F32, tag="a")
            nc.vector.tensor_copy(a_cur[:], em_e[:, 0:BL])
            b_cur = crf.tile([K, BL], F32, tag="b")
            nc.vector.tensor_copy(b_cur[:], em_e[:, (T - 1) * BL:T * BL])

            for s in range(1, MID + 1):
                aps = crfps.tile([K, BL], F32, tag="aps")
                nc.tensor.matmul(aps[:], ae_t[:], a_cur[:], start=True, stop=True)
                a_new = crf.tile([K, BL], F32, tag="a")
                nc.vector.tensor_tensor(a_new[:], aps[:],
                                        em_e[:, s * BL:(s + 1) * BL], ALU.mult)
                a_cur = a_new

                if s <= MID - 1:
                    t_b = T - 1 - s
                    bps = crfps.tile([K, BL], F32, tag="bps")
                    nc.tensor.matmul(bps[:], aet_t[:], b_cur[:], start=True, stop=True)
                    b_new = crf.tile([K, BL], F32, tag="b")
                    if t_b == MID:
                        nc.vector.tensor_copy(b_new[:], bps[:])
                    else:
                        nc.vector.tensor_tensor(b_new[:], bps[:],
                                                em_e[:, t_b * BL:(t_b + 1) * BL],
                                                ALU.mult)
                    b_cur = b_new

            prod = crf.tile([K, BL], F32, tag="prod")
            nc.vector.tensor_tensor(prod[:], a_cur[:], b_cur[:], ALU.mult)
            sps = crfps.tile([1, BL], F32, tag="aps")
            nc.tensor.matmul(sps[:], ones_t[:], prod[:], start=True, stop=True)
            logz = crf.tile([1, BL], F32, tag="logz")
            nc.scalar.activation(logz[:], sps[:], AF.Ln)
            logz2 = crf.tile([1, BL], F32, tag="logz2")
            nc.vector.tensor_scalar(logz2[:], logz[:], float(T * DELTA), None, ALU.add)
            nc.sync.dma_start(out[:], logz2[:])

    nc.compile()
    return nc


def _gate_permute(w):
    """Reorder gate blocks from reference (i,f,g,o) to kernel (i,f,o,g) and
    pre-double the g block so tanh(g) = 2*sigmoid(2g)-1 needs only sigmoid."""
    i, f, g, o = np.split(w, 4, axis=-1)
    return np.concatenate([i, f, o, 2.0 * g], axis=-1)


_PROGRAM_CACHE = {}


def kernel(tokens, emb, Wk_f, Wr_f, b_f, Wk_b, Wr_b, b_b, crf_kernel, crf_bias, trans):
    from concourse.bass_utils import run_bass_kernel_spmd

    tokens = np.asarray(tokens)
    emb = np.asarray(emb, dtype=np.float32)
    Wk_f = np.asarray(Wk_f, np.float32); Wr_f = np.asarray(Wr_f, np.float32)
    Wk_b = np.asarray(Wk_b, np.float32); Wr_b = np.asarray(Wr_b, np.float32)
    b_f = np.asarray(b_f, np.float32); b_b = np.asarray(b_b, np.float32)
    crf_kernel = np.asarray(crf_kernel, np.float32)
    crf_bias = np.asarray(crf_bias, np.float32)
    trans = np.asarray(trans, np.float32)

    if "nc" not in _PROGRAM_CACHE:
        _PROGRAM_CACHE["nc"] = _build_program()
    nc = _PROGRAM_CACHE["nc"]

    # ---- host staging ----
    emb_aug = np.concatenate(
        [emb, np.ones((V, 1), np.float32), np.zeros((V, EA - E - 1), np.float32)], 1)
    wk_aug_f = np.concatenate([Wk_f, b_f[None], np.zeros((EA - E - 1, G4), np.float32)], 0)
    wk_aug_b = np.concatenate([Wk_b, b_b[None], np.zeros((EA - E - 1, G4), np.float32)], 0)
    Ae = np.exp(trans).astype(np.float32)

    shared = {
        "emb": emb_aug,
        "wk_f": np.ascontiguousarray(_gate_permute(wk_aug_f)).astype(NPBF16),
        "wk_b": np.ascontiguousarray(_gate_permute(wk_aug_b)).astype(NPBF16),
        "wr_f": np.ascontiguousarray(_gate_permute(Wr_f)).astype(NPBF16),
        "wr_b": np.ascontiguousarray(_gate_permute(Wr_b)).astype(NPBF16),
        "ck_f": np.ascontiguousarray(crf_kernel[:U]).astype(NPBF16),
        "ck_b": np.ascontiguousarray(crf_kernel[U:]).astype(NPBF16),
        "ae": np.ascontiguousarray(Ae),
        "aet": np.ascontiguousarray(Ae.T),
        "embias": (crf_bias - DELTA).astype(np.float32).reshape(K, 1),
        "ident": np.eye(128, dtype=np.float32),
    }

    NBLK = T * BL // 128
    in_maps = []
    for c in range(NCORES):
        flat = tokens[c * BL:(c + 1) * BL].T.reshape(-1).astype(np.int32)  # t-major
        tok = np.ascontiguousarray(flat.reshape(NBLK, 128).T)
        in_maps.append({"tok": tok, **shared})

    res = run_bass_kernel_spmd(nc, in_maps, core_ids=list(range(NCORES)))
    outs = [res.results[c]["out"].reshape(BL).astype(np.float32) for c in range(NCORES)]
    return np.concatenate(outs, axis=0)


# revision 16
# speedup vs baseline: 1.0049x; 1.0049x over previous
"""Trainium2 Bass kernel for nn_LstmCrf: bidirectional LSTM + CRF log-partition.

Contract: kernel(**inputs) takes the FULL unsharded inputs (see shapes below) and
returns the FULL output logZ [128] f32. Internally shards the batch (128 rows)
across 8 NeuronCores (16 rows each), runs one SPMD Bass/Tile program, and
concatenates the per-core results.

Problem shapes (hardcoded): B=128, T=512, V=50000, E=100, U=128, K=32.

Per-core device program:
  1. Embedding gather via indirect DMA (tokens staged t-major), PE-transpose to
     x_T [104, T*16] bf16 (E padded to 104; col 100 carries 1.0 so the LSTM bias
     rides row 100 of the augmented Wk).
  2. Bidirectional LSTM scans, fwd+bwd interleaved per step; gates via one
     sigmoid + one tanh ACT op per step (gate blocks pre-permuted to i,f,o,g);
     h stored bf16.
  3. Emissions em = h_f@Ck_f + h_b@Ck_b; em_e = exp(em + crf_bias - delta) bf16.
  4. CRF forward DP in the exp domain (alpha_t = (Ae^T alpha) * em_e_t with
     Ae = exp(trans)), run meet-in-the-middle from both ends;
     logZ = log(sum_j alpha_mid * beta_mid) + T*delta,  delta = log(K).
"""
import sys
from contextlib import ExitStack

import numpy as np

for p in ("/opt/trn_rl_repo", "/root/.axon_site/_ro/trn_rl_repo"):
    if p not in sys.path:
        sys.path.append(p)

import ml_dtypes

NPBF16 = ml_dtypes.bfloat16

B, T = 128, 512
V, E, U, K = 50000, 100, 128, 32
NCORES = 8
BL = B // NCORES          # 16 rows per core
EA = 104                  # padded embedding dim
G4 = 4 * U
DELTA = float(np.log(K))


def _build_program(T=T):
    import concourse.bacc as bacc
    import concourse.bass as bass
    import concourse.mybir as mybir
    import concourse.tile as tile

    F32 = mybir.dt.float32
    BF16 = mybir.dt.bfloat16
    I32 = mybir.dt.int32
    AF = mybir.ActivationFunctionType
    ALU = mybir.AluOpType

    NBLK = T * BL // 128
    MID = T // 2

    nc = bacc.Bacc(None, target_bir_lowering=False, debug=False)

    tok = nc.dram_tensor("tok", [128, NBLK], I32, kind="ExternalInput")
    emb = nc.dram_tensor("emb", [V, EA], F32, kind="ExternalInput")
    wk_f = nc.dram_tensor("wk_f", [EA, G4], BF16, kind="ExternalInput")
    wk_b = nc.dram_tensor("wk_b", [EA, G4], BF16, kind="ExternalInput")
    wr_f = nc.dram_tensor("wr_f", [U, G4], BF16, kind="ExternalInput")
    wr_b = nc.dram_tensor("wr_b", [U, G4], BF16, kind="ExternalInput")
    ck_f = nc.dram_tensor("ck_f", [U, K], BF16, kind="ExternalInput")
    ck_b = nc.dram_tensor("ck_b", [U, K], BF16, kind="ExternalInput")
    ae = nc.dram_tensor("ae", [K, K], F32, kind="ExternalInput")
    aet = nc.dram_tensor("aet", [K, K], F32, kind="ExternalInput")
    embias = nc.dram_tensor("embias", [K, 1], F32, kind="ExternalInput")
    ident = nc.dram_tensor("ident", [128, 128], F32, kind="ExternalInput")
    out = nc.dram_tensor("out", [1, BL], F32, kind="ExternalOutput")

    def block_order(nblk):
        order = []
        lo, hi = 0, nblk - 1
        while lo <= hi:
            order.append(lo)
            if hi != lo:
                order.append(hi)
            lo += 1
            hi -= 1
        return order

    with tile.TileContext(nc) as tc, ExitStack() as ctx:
        P = ctx.enter_context(tc.tile_pool(name="persist", bufs=1))
        tok_t = P.tile([128, NBLK], I32, tag="tok")
        wkf_t = P.tile([EA, G4], BF16, tag="wkf")
        wkb_t = P.tile([EA, G4], BF16, tag="wkb")
        wrf_t = P.tile([U, G4], BF16, tag="wrf")
        wrb_t = P.tile([U, G4], BF16, tag="wrb")
        ckf_t = P.tile([U, K], BF16, tag="ckf")
        ckb_t = P.tile([U, K], BF16, tag="ckb")
        ae_t = P.tile([K, K], F32, tag="ae")
        aet_t = P.tile([K, K], F32, tag="aet")
        embias_t = P.tile([K, 1], F32, tag="embias")
        ident_t = P.tile([128, 128], F32, tag="ident")
        xT = P.tile([EA, T * BL], BF16, tag="xT")
        h_all = P.tile([U, 2 * T * BL], BF16, tag="hall")
        em_e = P.tile([K, T * BL], BF16, tag="eme")
        ones_t = P.tile([K, 1], F32, tag="ones")

        nc.sync.dma_start(tok_t[:], tok[:])
        nc.sync.dma_start(wkf_t[:], wk_f[:])
        nc.sync.dma_start(wkb_t[:], wk_b[:])
        nc.sync.dma_start(wrf_t[:], wr_f[:])
        nc.sync.dma_start(wrb_t[:], wr_b[:])
        nc.sync.dma_start(ckf_t[:], ck_f[:])
        nc.sync.dma_start(ckb_t[:], ck_b[:])
        nc.sync.dma_start(ae_t[:], ae[:])
        nc.sync.dma_start(aet_t[:], aet[:])
        nc.sync.dma_start(embias_t[:], embias[:])
        nc.sync.dma_start(ident_t[:], ident[:])
        nc.vector.memset(ones_t[:], 1.0)

        with (
            tc.tile_pool(name="gat", bufs=4) as gat,
            tc.tile_pool(name="tp_ps", bufs=2, space="PSUM") as tp_ps,
            tc.tile_pool(name="zps", bufs=4, space="PSUM") as zps,
            tc.tile_pool(name="sg", bufs=3) as sgp,
            tc.tile_pool(name="cst", bufs=3) as cst,
        ):
            for k in block_order(NBLK):
                g = gat.tile([128, EA], F32, tag="g")
                nc.gpsimd.indirect_dma_start(
                    out=g[:],
                    out_offset=None,
                    in_=emb[:],
                    in_offset=bass.IndirectOffsetOnAxis(ap=tok_t[:, k:k + 1], axis=0),
                )
                pt = tp_ps.tile([EA, 128], F32, tag="pt")
                nc.tensor.transpose(pt[:], g[:], ident_t[:])
                nc.vector.tensor_copy(xT[:, k * 128:(k + 1) * 128], pt[:])

            # LSTM scans.
            # PSUM z layout per step: [i_f f_f o_f g2_f | i_b f_b o_b g2_b]
            # (g2 = pre-doubled g gate; host scaled its weights by 2).
            # sg = sigmoid(z) on all 128 cols in ONE ACT op; tanh(g) = 2*sg(g2)-1.
            # State tile X_t [128, 2, 32] per dir: [tg_t (16) | c_{t-1} (16)].
            # prods = sg[i|f] * [tg | c]; c_t = prods[:16] + prods[16:32].
            c_prev = None
            for t in range(T):
                z = zps.tile([128, 128], F32, tag="z")
                # emit all x-projection MMs first: they depend only on xT, so
                # the PE FIFO can run them during the previous step's ACT/DVE
                # phase instead of stalling them behind h-dependent Wr MMs.
                # Gate-major z layout: gate g at cols [g*32,(g+1)*32), fwd dir
                # at +0, bwd at +16 -> sigma slices are contiguous [128,32].
                # One accumulation group per z tile: start=True on the FIRST MM
                # zeroes the whole 2KB bank; everything else accumulates.
                # x-projection MMs are emitted first so the PE FIFO runs them
                # during the previous step's ACT/DVE phase.
                first = True
                for d, wk_t in ((0, wkf_t), (1, wkb_t)):
                    tt = t if d == 0 else T - 1 - t
                    xs = xT[:, tt * BL:(tt + 1) * BL]
                    for gi in range(4):
                        oc = gi * 32 + d * BL
                        nc.tensor.matmul(
                            z[:, oc:oc + BL],
                            wk_t[:, gi * U:(gi + 1) * U],
                            xs,
                            start=first,
                            stop=(t == 0 and d == 1 and gi == 3),
                        )
                        first = False
                if t > 0:
                    for d, (wr_t, hofs) in ((0, (wrf_t, 0)), (1, (wrb_t, T * BL))):
                        hprev = t - 1 if d == 0 else T - t
                        hs = h_all[:, hofs + hprev * BL:hofs + (hprev + 1) * BL]
                        for gi in range(4):
                            oc = gi * 32 + d * BL
                            nc.tensor.matmul(
                                z[:, oc:oc + BL],
                                wr_t[:, gi * U:(gi + 1) * U],
                                hs,
                                start=False,
                                stop=(d == 1 and gi == 3),
                            )
                sg = sgp.tile([128, 128], F32, tag="sg")
                nc.scalar.activation(sg[:], z[:], AF.Sigmoid)
                # si = sg[0:32], sf = sg[32:64], so = sg[64:96], sgg = sg[96:128]
                # c = sf*c_prev + si*tanh(g), tanh(g) = 2*sg(g2)-1:
                #   m1 = si*sgg; m2 = sf*c_prev; m3 = m2 - si; c = 2*m1 + m3
                # c = sf*c_prev + si*(2*sg(g2)-1):
                #   m1 = si*sgg; m2 = sf*c_prev; w = 2*m1 - si; c = w + m2
                # (m1, m2 independent; w depends on m1 two issues back -> only
                #  the final add pays a same-engine RAW stall)
                m1 = cst.tile([128, 32], F32, tag="m1")
                nc.vector.tensor_tensor(m1[:], sg[:, 0:32], sg[:, 96:128], ALU.mult)
                if t == 0:
                    c_new = cst.tile([128, 32], F32, tag="c")
                    nc.vector.scalar_tensor_tensor(
                        c_new[:], m1[:], 2.0, sg[:, 0:32], ALU.mult, ALU.subtract)
                else:
                    m2 = cst.tile([128, 32], F32, tag="m2")
                    nc.vector.tensor_tensor(m2[:], sg[:, 32:64], c_prev[:], ALU.mult)
                    w = cst.tile([128, 32], F32, tag="w")
                    nc.vector.scalar_tensor_tensor(
                        w[:], m1[:], 2.0, sg[:, 0:32], ALU.mult, ALU.subtract)
                    c_new = cst.tile([128, 32], F32, tag="c")
                    nc.vector.tensor_tensor(c_new[:], w[:], m2[:], ALU.add)
                c_prev = c_new
                tct = cst.tile([128, 32], F32, tag="tc")
                nc.scalar.activation(tct[:], c_new[:], AF.Tanh)
                for d, hofs in ((0, 0), (1, T * BL)):
                    tt = t if d == 0 else T - 1 - t
                    nc.vector.tensor_tensor(
                        h_all[:, hofs + tt * BL:hofs + (tt + 1) * BL],
                        sg[:, 64 + d * BL:64 + d * BL + BL],
                        tct[:, d * BL:d * BL + BL], ALU.mult,
                    )

        # keep the exp/ln table phase strictly after the sigmoid/tanh phase
        tc.no_sync_barrier()

        EMC = 512
        with (
            tc.tile_pool(name="emps", bufs=4, space="PSUM") as emps,
            tc.tile_pool(name="crf", bufs=3) as crf,
            tc.tile_pool(name="crfps", bufs=2, space="PSUM") as crfps,
        ):
            for ch in range(T * BL // EMC):
                ep = emps.tile([K, EMC], F32, tag="ep")
                nc.tensor.matmul(ep[:], ckf_t[:], h_all[:, ch * EMC:(ch + 1) * EMC],
                                 start=True, stop=False)
                nc.tensor.matmul(ep[:], ckb_t[:],
                                 h_all[:, T * BL + ch * EMC:T * BL + (ch + 1) * EMC],
                                 start=False, stop=True)
                nc.scalar.activation(em_e[:, ch * EMC:(ch + 1) * EMC], ep[:],
                                     AF.Exp, bias=embias_t[:], scale=1.0)

            a_cur = crf.tile([K, BL], F32, tag="a")
            nc.vector.tensor_copy(a_cur[:], em_e[:, 0:BL])
            b_cur = crf.tile([K, BL], F32, tag="b")
            nc.vector.tensor_copy(b_cur[:], em_e[:, (T - 1) * BL:T * BL])

            for s in range(1, MID + 1):
                aps = crfps.tile([K, BL], F32, tag="aps")
                nc.tensor.matmul(aps[:], ae_t[:], a_cur[:], start=True, stop=True)
                a_new = crf.tile([K, BL], F32, tag="a")
                nc.vector.tensor_tensor(a_new[:], aps[:],
                                        em_e[:, s * BL:(s + 1) * BL], ALU.mult)
                a_cur = a_new

                if s <= MID - 1:
                    t_b = T - 1 - s
                    bps = crfps.tile([K, BL], F32, tag="bps")
                    nc.tensor.matmul(bps[:], aet_t[:], b_cur[:], start=True, stop=True)
                    b_new = crf.tile([K, BL], F32, tag="b")
                    if t_b == MID:
                        nc.vector.tensor_copy(b_new[:], bps[:])
                    else:
                        nc.vector.tensor_tensor(b_new[:], bps[:],
                                                em_e[:, t_b * BL:(t_b + 1) * BL],
                                                ALU.mult)
                    b_cur = b_new

            prod = crf.tile([K, BL], F32, tag="prod")
            nc.vector.tensor_tensor(prod[:], a_cur[:], b_cur[:], ALU.mult)
            sps = crfps.tile([1, BL], F32, tag="aps")
            nc.tensor.matmul(sps[:], ones_t[:], prod[:], start=True, stop=True)
            logz = crf.tile([1, BL], F32, tag="logz")
            nc.scalar.activation(logz[:], sps[:], AF.Ln)
            logz2 = crf.tile([1, BL], F32, tag="logz2")
            nc.vector.tensor_scalar(logz2[:], logz[:], float(T * DELTA), None, ALU.add)
            nc.sync.dma_start(out[:], logz2[:])

    nc.compile()
    return nc


def _gate_permute(w):
    """Reorder gate blocks from reference (i,f,g,o) to kernel (i,f,o,g) and
    pre-double the g block so tanh(g) = 2*sigmoid(2g)-1 needs only sigmoid."""
    i, f, g, o = np.split(w, 4, axis=-1)
    return np.concatenate([i, f, o, 2.0 * g], axis=-1)


_PROGRAM_CACHE = {}


def kernel(tokens, emb, Wk_f, Wr_f, b_f, Wk_b, Wr_b, b_b, crf_kernel, crf_bias, trans):
    from concourse.bass_utils import run_bass_kernel_spmd

    tokens = np.asarray(tokens)
    emb = np.asarray(emb, dtype=np.float32)
    Wk_f = np.asarray(Wk_f, np.float32); Wr_f = np.asarray(Wr_f, np.float32)
    Wk_b = np.asarray(Wk_b, np.float32); Wr_b = np.asarray(Wr_b, np.float32)
    b_f = np.asarray(b_f, np.float32); b_b = np.asarray(b_b, np.float32)
    crf_kernel = np.asarray(crf_kernel, np.float32)
    crf_bias = np.asarray(crf_bias, np.float32)
    trans = np.asarray(trans, np.float32)

    if "nc" not in _PROGRAM_CACHE:
        _PROGRAM_CACHE["nc"] = _build_program()
    nc = _PROGRAM_CACHE["nc"]

    # ---- host staging ----
    emb_aug = np.concatenate(
        [emb, np.ones((V, 1), np.float32), np.zeros((V, EA - E - 1), np.float32)], 1)
    wk_aug_f = np.concatenate([Wk_f, b_f[None], np.zeros((EA - E - 1, G4), np.float32)], 0)
    wk_aug_b = np.concatenate([Wk_b, b_b[None], np.zeros((EA - E - 1, G4), np.float32)], 0)
    Ae = np.exp(trans).astype(np.float32)

    shared = {
        "emb": emb_aug,
        "wk_f": np.ascontiguousarray(_gate_permute(wk_aug_f)).astype(NPBF16),
        "wk_b": np.ascontiguousarray(_gate_permute(wk_aug_b)).astype(NPBF16),
        "wr_f": np.ascontiguousarray(_gate_permute(Wr_f)).astype(NPBF16),
        "wr_b": np.ascontiguousarray(_gate_permute(Wr_b)).astype(NPBF16),
        "ck_f": np.ascontiguousarray(crf_kernel[:U]).astype(NPBF16),
        "ck_b": np.ascontiguousarray(crf_kernel[U:]).astype(NPBF16),
        "ae": np.ascontiguousarray(Ae),
        "aet": np.ascontiguousarray(Ae.T),
        "embias": (crf_bias - DELTA).astype(np.float32).reshape(K, 1),
        "ident": np.eye(128, dtype=np.float32),
    }

    NBLK = T * BL // 128
    in_maps = []
    for c in range(NCORES):
        flat = tokens[c * BL:(c + 1) * BL].T.reshape(-1).astype(np.int32)  # t-major
        tok = np.ascontiguousarray(flat.reshape(NBLK, 128).T)
        in_maps.append({"tok": tok, **shared})

    res = run_bass_kernel_spmd(nc, in_maps, core_ids=list(range(NCORES)))
    outs = [res.results[c]["out"].reshape(BL).astype(np.float32) for c in range(NCORES)]
    return np.concatenate(outs, axis=0)


# revision 17
# speedup vs baseline: 1.0059x; 1.0010x over previous
"""Trainium2 Bass kernel for nn_LstmCrf: bidirectional LSTM + CRF log-partition.

Contract: kernel(**inputs) takes the FULL unsharded inputs (see shapes below) and
returns the FULL output logZ [128] f32. Internally shards the batch (128 rows)
across 8 NeuronCores (16 rows each), runs one SPMD Bass/Tile program, and
concatenates the per-core results.

Problem shapes (hardcoded): B=128, T=512, V=50000, E=100, U=128, K=32.

Per-core device program:
  1. Embedding gather via indirect DMA (tokens staged t-major), PE-transpose to
     x_T [104, T*16] bf16 (E padded to 104; col 100 carries 1.0 so the LSTM bias
     rides row 100 of the augmented Wk).
  2. Bidirectional LSTM scans, fwd+bwd interleaved per step; gates via one
     sigmoid + one tanh ACT op per step (gate blocks pre-permuted to i,f,o,g);
     h stored bf16.
  3. Emissions em = h_f@Ck_f + h_b@Ck_b; em_e = exp(em + crf_bias - delta) bf16.
  4. CRF forward DP in the exp domain (alpha_t = (Ae^T alpha) * em_e_t with
     Ae = exp(trans)), run meet-in-the-middle from both ends;
     logZ = log(sum_j alpha_mid * beta_mid) + T*delta,  delta = log(K).
"""
import sys
from contextlib import ExitStack

import numpy as np

for p in ("/opt/trn_rl_repo", "/root/.axon_site/_ro/trn_rl_repo"):
    if p not in sys.path:
        sys.path.append(p)

import ml_dtypes

NPBF16 = ml_dtypes.bfloat16

B, T = 128, 512
V, E, U, K = 50000, 100, 128, 32
NCORES = 8
BL = B // NCORES          # 16 rows per core
EA = 104                  # padded embedding dim
G4 = 4 * U
DELTA = float(np.log(K))


def _build_program(T=T):
    import concourse.bacc as bacc
    import concourse.bass as bass
    import concourse.mybir as mybir
    import concourse.tile as tile

    F32 = mybir.dt.float32
    BF16 = mybir.dt.bfloat16
    I32 = mybir.dt.int32
    AF = mybir.ActivationFunctionType
    ALU = mybir.AluOpType

    NBLK = T * BL // 128
    MID = T // 2

    nc = bacc.Bacc(None, target_bir_lowering=False, debug=False)

    tok = nc.dram_tensor("tok", [128, NBLK], I32, kind="ExternalInput")
    emb = nc.dram_tensor("emb", [V, EA], F32, kind="ExternalInput")
    wk_f = nc.dram_tensor("wk_f", [EA, G4], BF16, kind="ExternalInput")
    wk_b = nc.dram_tensor("wk_b", [EA, G4], BF16, kind="ExternalInput")
    wr_f = nc.dram_tensor("wr_f", [U, G4], BF16, kind="ExternalInput")
    wr_b = nc.dram_tensor("wr_b", [U, G4], BF16, kind="ExternalInput")
    ck_f = nc.dram_tensor("ck_f", [U, K], BF16, kind="ExternalInput")
    ck_b = nc.dram_tensor("ck_b", [U, K], BF16, kind="ExternalInput")
    ae = nc.dram_tensor("ae", [K, K], F32, kind="ExternalInput")
    aet = nc.dram_tensor("aet", [K, K], F32, kind="ExternalInput")
    embias = nc.dram_tensor("embias", [K, 1], F32, kind="ExternalInput")
    ident = nc.dram_tensor("ident", [128, 128], F32, kind="ExternalInput")
    out = nc.dram_tensor("out", [1, BL], F32, kind="ExternalOutput")

    def block_order(nblk):
        order = []
        lo, hi = 0, nblk - 1
        while lo <= hi:
            order.append(lo)
            if hi != lo:
                order.append(hi)
            lo += 1
            hi -= 1
        return order

    with tile.TileContext(nc) as tc, ExitStack() as ctx:
        P = ctx.enter_context(tc.tile_pool(name="persist", bufs=1))
        tok_t = P.tile([128, NBLK], I32, tag="tok")
        wkf_t = P.tile([EA, G4], BF16, tag="wkf")
        wkb_t = P.tile([EA, G4], BF16, tag="wkb")
        wrf_t = P.tile([U, G4], BF16, tag="wrf")
        wrb_t = P.tile([U, G4], BF16, tag="wrb")
        ckf_t = P.tile([U, K], BF16, tag="ckf")
        ckb_t = P.tile([U, K], BF16, tag="ckb")
        ae_t = P.tile([K, K], F32, tag="ae")
        aet_t = P.tile([K, K], F32, tag="aet")
        embias_t = P.tile([K, 1], F32, tag="embias")
        ident_t = P.tile([128, 128], F32, tag="ident")
        xT = P.tile([EA, T * BL], BF16, tag="xT")
        h_all = P.tile([U, 2 * T * BL], BF16, tag="hall")
        em_e = P.tile([K, T * BL], BF16, tag="eme")
        ones_t = P.tile([K, 1], F32, tag="ones")

        nc.sync.dma_start(tok_t[:], tok[:])
        nc.sync.dma_start(wkf_t[:], wk_f[:])
        nc.sync.dma_start(wkb_t[:], wk_b[:])
        nc.sync.dma_start(wrf_t[:], wr_f[:])
        nc.sync.dma_start(wrb_t[:], wr_b[:])
        nc.sync.dma_start(ckf_t[:], ck_f[:])
        nc.sync.dma_start(ckb_t[:], ck_b[:])
        nc.sync.dma_start(ae_t[:], ae[:])
        nc.sync.dma_start(aet_t[:], aet[:])
        nc.sync.dma_start(embias_t[:], embias[:])
        nc.sync.dma_start(ident_t[:], ident[:])
        nc.vector.memset(ones_t[:], 1.0)

        with (
            tc.tile_pool(name="gat", bufs=4) as gat,
            tc.tile_pool(name="tp_ps", bufs=2, space="PSUM") as tp_ps,
            tc.tile_pool(name="zps", bufs=4, space="PSUM") as zps,
            tc.tile_pool(name="sg", bufs=3) as sgp,
            tc.tile_pool(name="cst", bufs=3) as cst,
        ):
            order = block_order(NBLK)

            def emit_block(k):
                g = gat.tile([128, EA], F32, tag="g")
                nc.gpsimd.indirect_dma_start(
                    out=g[:],
                    out_offset=None,
                    in_=emb[:],
                    in_offset=bass.IndirectOffsetOnAxis(ap=tok_t[:, k:k + 1], axis=0),
                )
                pt = tp_ps.tile([EA, 128], F32, tag="pt")
                nc.tensor.transpose(pt[:], g[:], ident_t[:])
                nc.vector.tensor_copy(xT[:, k * 128:(k + 1) * 128], pt[:])

            # Pace the gather: the scan consumes one lo/hi block pair per 8
            # steps; emit blocks inside the loop with 3 pairs of lookahead so
            # the gather work interleaves into engine slack instead of
            # congesting the FIFOs during the first ~20 steps.
            oi = 0
            while oi < min(NBLK, 6):
                emit_block(order[oi])
                oi += 1

            # LSTM scans.
            # PSUM z layout per step: [i_f f_f o_f g2_f | i_b f_b o_b g2_b]
            # (g2 = pre-doubled g gate; host scaled its weights by 2).
            # sg = sigmoid(z) on all 128 cols in ONE ACT op; tanh(g) = 2*sg(g2)-1.
            # State tile X_t [128, 2, 32] per dir: [tg_t (16) | c_{t-1} (16)].
            # prods = sg[i|f] * [tg | c]; c_t = prods[:16] + prods[16:32].
            c_prev = None
            for t in range(T):
                if t % 8 == 0:
                    target = min(NBLK, 2 * (t // 8 + 3))
                    while oi < target:
                        emit_block(order[oi])
                        oi += 1
                z = zps.tile([128, 128], F32, tag="z")
                # emit all x-projection MMs first: they depend only on xT, so
                # the PE FIFO can run them during the previous step's ACT/DVE
                # phase instead of stalling them behind h-dependent Wr MMs.
                # Gate-major z layout: gate g at cols [g*32,(g+1)*32), fwd dir
                # at +0, bwd at +16 -> sigma slices are contiguous [128,32].
                # One accumulation group per z tile: start=True on the FIRST MM
                # zeroes the whole 2KB bank; everything else accumulates.
                # x-projection MMs are emitted first so the PE FIFO runs them
                # during the previous step's ACT/DVE phase.
                first = True
                for d, wk_t in ((0, wkf_t), (1, wkb_t)):
                    tt = t if d == 0 else T - 1 - t
                    xs = xT[:, tt * BL:(tt + 1) * BL]
                    for gi in range(4):
                        oc = gi * 32 + d * BL
                        nc.tensor.matmul(
                            z[:, oc:oc + BL],
                            wk_t[:, gi * U:(gi + 1) * U],
                            xs,
                            start=first,
                            stop=(t == 0 and d == 1 and gi == 3),
                        )
                        first = False
                if t > 0:
                    for d, (wr_t, hofs) in ((0, (wrf_t, 0)), (1, (wrb_t, T * BL))):
                        hprev = t - 1 if d == 0 else T - t
                        hs = h_all[:, hofs + hprev * BL:hofs + (hprev + 1) * BL]
                        for gi in range(4):
                            oc = gi * 32 + d * BL
                            nc.tensor.matmul(
                                z[:, oc:oc + BL],
                                wr_t[:, gi * U:(gi + 1) * U],
                                hs,
                                start=False,
                                stop=(d == 1 and gi == 3),
                            )
                sg = sgp.tile([128, 128], F32, tag="sg")
                nc.scalar.activation(sg[:], z[:], AF.Sigmoid)
                # si = sg[0:32], sf = sg[32:64], so = sg[64:96], sgg = sg[96:128]
                # c = sf*c_prev + si*tanh(g), tanh(g) = 2*sg(g2)-1:
                #   m1 = si*sgg; m2 = sf*c_prev; m3 = m2 - si; c = 2*m1 + m3
                # c = sf*c_prev + si*(2*sg(g2)-1):
                #   m1 = si*sgg; m2 = sf*c_prev; w = 2*m1 - si; c = w + m2
                # (m1, m2 independent; w depends on m1 two issues back -> only
                #  the final add pays a same-engine RAW stall)
                m1 = cst.tile([128, 32], F32, tag="m1")
                nc.vector.tensor_tensor(m1[:], sg[:, 0:32], sg[:, 96:128], ALU.mult)
                if t == 0:
                    c_new = cst.tile([128, 32], F32, tag="c")
                    nc.vector.scalar_tensor_tensor(
                        c_new[:], m1[:], 2.0, sg[:, 0:32], ALU.mult, ALU.subtract)
                else:
                    m2 = cst.tile([128, 32], F32, tag="m2")
                    nc.vector.tensor_tensor(m2[:], sg[:, 32:64], c_prev[:], ALU.mult)
                    w = cst.tile([128, 32], F32, tag="w")
                    nc.vector.scalar_tensor_tensor(
                        w[:], m1[:], 2.0, sg[:, 0:32], ALU.mult, ALU.subtract)
                    c_new = cst.tile([128, 32], F32, tag="c")
                    nc.vector.tensor_tensor(c_new[:], w[:], m2[:], ALU.add)
                c_prev = c_new
                tct = cst.tile([128, 32], F32, tag="tc")
                nc.scalar.activation(tct[:], c_new[:], AF.Tanh)
                for d, hofs in ((0, 0), (1, T * BL)):
                    tt = t if d == 0 else T - 1 - t
                    nc.vector.tensor_tensor(
                        h_all[:, hofs + tt * BL:hofs + (tt + 1) * BL],
                        sg[:, 64 + d * BL:64 + d * BL + BL],
                        tct[:, d * BL:d * BL + BL], ALU.mult,
                    )

        # keep the exp/ln table phase strictly after the sigmoid/tanh phase
        tc.no_sync_barrier()

        EMC = 512
        with (
            tc.tile_pool(name="emps", bufs=4, space="PSUM") as emps,
            tc.tile_pool(name="crf", bufs=3) as crf,
            tc.tile_pool(name="crfps", bufs=2, space="PSUM") as crfps,
        ):
            for ch in range(T * BL // EMC):
                ep = emps.tile([K, EMC], F32, tag="ep")
                nc.tensor.matmul(ep[:], ckf_t[:], h_all[:, ch * EMC:(ch + 1) * EMC],
                                 start=True, stop=False)
                nc.tensor.matmul(ep[:], ckb_t[:],
                                 h_all[:, T * BL + ch * EMC:T * BL + (ch + 1) * EMC],
                                 start=False, stop=True)
                nc.scalar.activation(em_e[:, ch * EMC:(ch + 1) * EMC], ep[:],
                                     AF.Exp, bias=embias_t[:], scale=1.0)

            a_cur = crf.tile([K, BL], F32, tag="a")
            nc.vector.tensor_copy(a_cur[:], em_e[:, 0:BL])
            b_cur = crf.tile([K, BL], F32, tag="b")
            nc.vector.tensor_copy(b_cur[:], em_e[:, (T - 1) * BL:T * BL])

            for s in range(1, MID + 1):
                aps = crfps.tile([K, BL], F32, tag="aps")
                nc.tensor.matmul(aps[:], ae_t[:], a_cur[:], start=True, stop=True)
                a_new = crf.tile([K, BL], F32, tag="a")
                nc.vector.tensor_tensor(a_new[:], aps[:],
                                        em_e[:, s * BL:(s + 1) * BL], ALU.mult)
                a_cur = a_new

                if s <= MID - 1:
                    t_b = T - 1 - s
                    bps = crfps.tile([K, BL], F32, tag="bps")
                    nc.tensor.matmul(bps[:], aet_t[:], b_cur[:], start=True, stop=True)
                    b_new = crf.tile([K, BL], F32, tag="b")
                    if t_b == MID:
                        nc.vector.tensor_copy(b_new[:], bps[:])
                    else:
                        nc.vector.tensor_tensor(b_new[:], bps[:],
                                                em_e[:, t_b * BL:(t_b + 1) * BL],
                                                ALU.mult)
                    b_cur = b_new

            prod = crf.tile([K, BL], F32, tag="prod")
            nc.vector.tensor_tensor(prod[:], a_cur[:], b_cur[:], ALU.mult)
            sps = crfps.tile([1, BL], F32, tag="aps")
            nc.tensor.matmul(sps[:], ones_t[:], prod[:], start=True, stop=True)
            logz = crf.tile([1, BL], F32, tag="logz")
            nc.scalar.activation(logz[:], sps[:], AF.Ln)
            logz2 = crf.tile([1, BL], F32, tag="logz2")
            nc.vector.tensor_scalar(logz2[:], logz[:], float(T * DELTA), None, ALU.add)
            nc.sync.dma_start(out[:], logz2[:])

    nc.compile()
    return nc


def _gate_permute(w):
    """Reorder gate blocks from reference (i,f,g,o) to kernel (i,f,o,g) and
    pre-double the g block so tanh(g) = 2*sigmoid(2g)-1 needs only sigmoid."""
    i, f, g, o = np.split(w, 4, axis=-1)
    return np.concatenate([i, f, o, 2.0 * g], axis=-1)


_PROGRAM_CACHE = {}


def kernel(tokens, emb, Wk_f, Wr_f, b_f, Wk_b, Wr_b, b_b, crf_kernel, crf_bias, trans):
    from concourse.bass_utils import run_bass_kernel_spmd

    tokens = np.asarray(tokens)
    emb = np.asarray(emb, dtype=np.float32)
    Wk_f = np.asarray(Wk_f, np.float32); Wr_f = np.asarray(Wr_f, np.float32)
    Wk_b = np.asarray(Wk_b, np.float32); Wr_b = np.asarray(Wr_b, np.float32)
    b_f = np.asarray(b_f, np.float32); b_b = np.asarray(b_b, np.float32)
    crf_kernel = np.asarray(crf_kernel, np.float32)
    crf_bias = np.asarray(crf_bias, np.float32)
    trans = np.asarray(trans, np.float32)

    if "nc" not in _PROGRAM_CACHE:
        _PROGRAM_CACHE["nc"] = _build_program()
    nc = _PROGRAM_CACHE["nc"]

    # ---- host staging ----
    emb_aug = np.concatenate(
        [emb, np.ones((V, 1), np.float32), np.zeros((V, EA - E - 1), np.float32)], 1)
    wk_aug_f = np.concatenate([Wk_f, b_f[None], np.zeros((EA - E - 1, G4), np.float32)], 0)
    wk_aug_b = np.concatenate([Wk_b, b_b[None], np.zeros((EA - E - 1, G4), np.float32)], 0)
    Ae = np.exp(trans).astype(np.float32)

    shared = {
        "emb": emb_aug,
        "wk_f": np.ascontiguousarray(_gate_permute(wk_aug_f)).astype(NPBF16),
        "wk_b": np.ascontiguousarray(_gate_permute(wk_aug_b)).astype(NPBF16),
        "wr_f": np.ascontiguousarray(_gate_permute(Wr_f)).astype(NPBF16),
        "wr_b": np.ascontiguousarray(_gate_permute(Wr_b)).astype(NPBF16),
        "ck_f": np.ascontiguousarray(crf_kernel[:U]).astype(NPBF16),
        "ck_b": np.ascontiguousarray(crf_kernel[U:]).astype(NPBF16),
        "ae": np.ascontiguousarray(Ae),
        "aet": np.ascontiguousarray(Ae.T),
        "embias": (crf_bias - DELTA).astype(np.float32).reshape(K, 1),
        "ident": np.eye(128, dtype=np.float32),
    }

    NBLK = T * BL // 128
    in_maps = []
    for c in range(NCORES):
        flat = tokens[c * BL:(c + 1) * BL].T.reshape(-1).astype(np.int32)  # t-major
        tok = np.ascontiguousarray(flat.reshape(NBLK, 128).T)
        in_maps.append({"tok": tok, **shared})

    res = run_bass_kernel_spmd(nc, in_maps, core_ids=list(range(NCORES)))
    outs = [res.results[c]["out"].reshape(BL).astype(np.float32) for c in range(NCORES)]
    return np.concatenate(outs, axis=0)
